# revision 6
# baseline (speedup 1.0000x reference)
"""Trainium2 Bass kernel for nn_CrossAttentionLayer (sigmoid cross-attention).

Sharding: pure data-parallel over the batch dim — core c computes batch c
(bs=8 across 8 NeuronCores, zero collectives).

Per-core device program (batch-local shapes: text (512,1024), av (1024,1024)):
  1. LayerNorm both streams in token-major layout (bn_stats/bn_aggr on DVE,
     rsqrt via ACT-Sqrt + DVE reciprocal, fused (x-mu)*rinv on DVE), bf16 out.
  2. Round-trip the normalized activations through DRAM and reload transposed
     via the DMA xbar (feature-major tiles for matmul contraction).
  3. Projections q/k/v as bf16 matmuls accumulating over 8 d-blocks in PSUM;
     LN affine params are folded into the weights host-side; projection biases
     applied on PSUM eviction (per-partition bias for qT/kT, K=1 ones-matmul
     for token-major v).
  4. Per-head sigmoid attention: S^T = kT_h^T @ qT_h (K=64), sigmoid(S/8) on
     ACT straight out of PSUM into bf16; out^T_h accumulated over kv blocks
     with v as the stationary operand; attention-mean over heads via a
     pairwise bf16 add tree split across DVE and GpSimd.
  5. Final transposes back to token-major via DMA xbar (DRAM round-trip) and
     SWDGE cast-stores (bf16 -> f32) for both outputs.
"""
import numpy as np
import ml_dtypes

import concourse.bacc as bacc
import concourse.mybir as mybir
import concourse.tile as tile
from concourse.bass_utils import run_bass_kernel_spmd

bf16 = ml_dtypes.bfloat16
BF = mybir.dt.bfloat16
F32 = mybir.dt.float32
AF = mybir.ActivationFunctionType
ALU = mybir.AluOpType

NW = 512      # num_word (queries)
NV = 1024     # num_valid (keys/values)
D = 1024      # d_model
H = 16        # heads
DK = 64       # head dim
NCORES = 8

_CACHE: dict = {}


def _build_program():
    nc = bacc.Bacc("TRN2", target_bir_lowering=False, debug=False)

    xt_d = nc.declare_dram_parameter("xt", [NW, D], BF, isOutput=False)
    xa_d = nc.declare_dram_parameter("xa", [NV, D], BF, isOutput=False)
    wq_d = nc.declare_dram_parameter("wqT", [D, D], BF, isOutput=False)
    wk_d = nc.declare_dram_parameter("wkT", [D, D], BF, isOutput=False)
    wv_d = nc.declare_dram_parameter("wvT", [D, D], BF, isOutput=False)
    bq_d = nc.declare_dram_parameter("bq", [D], F32, isOutput=False)
    bk_d = nc.declare_dram_parameter("bk", [D], F32, isOutput=False)
    bv_d = nc.declare_dram_parameter("bv", [1, D], BF, isOutput=False)

    out_d = nc.declare_dram_parameter("out", [NW, D], F32, isOutput=True)
    am_d = nc.declare_dram_parameter("am", [NW, NV], F32, isOutput=True)

    that_dram = nc.dram_tensor("that_scratch", [NW, D], BF)
    ahat_dram = nc.dram_tensor("ahat_scratch", [NV, D], BF)
    outT_dram = nc.dram_tensor("outT_scratch", [D, NW], BF)
    amT_dram = nc.dram_tensor("amT_scratch", [NV, NW], BF)

    with tile.TileContext(nc) as tc:
        import contextlib
        with contextlib.ExitStack() as ctx:
            const_p = ctx.enter_context(tc.tile_pool(name="const", bufs=1))
            in_p = ctx.enter_context(tc.tile_pool(name="in", bufs=3))
            stat_p = ctx.enter_context(tc.tile_pool(name="stat", bufs=24))
            hat_p = ctx.enter_context(tc.tile_pool(name="hat", bufs=3))
            tT_p = ctx.enter_context(tc.tile_pool(name="tT", bufs=8))
            aT_p = ctx.enter_context(tc.tile_pool(name="aT", bufs=8))
            w_p = ctx.enter_context(tc.tile_pool(name="w", bufs=12))
            qT_p = ctx.enter_context(tc.tile_pool(name="qT", bufs=8))
            kT_p = ctx.enter_context(tc.tile_pool(name="kT", bufs=8))
            v_p = ctx.enter_context(tc.tile_pool(name="v", bufs=8))
            pt_p = ctx.enter_context(tc.tile_pool(name="pt", bufs=18))
            mean_p = ctx.enter_context(tc.tile_pool(name="mean", bufs=36))
            outT_p = ctx.enter_context(tc.tile_pool(name="outT", bufs=8))
            amT_p = ctx.enter_context(tc.tile_pool(name="amT", bufs=8))
            fin_p = ctx.enter_context(tc.tile_pool(name="fin", bufs=3))

            eps_t = const_p.tile([128, 1], F32)
            nc.gpsimd.memset(eps_t[:], 1e-5)
            zero_t = const_p.tile([128, 1], F32)
            nc.gpsimd.memset(zero_t[:], 0.0)
            ones_t = const_p.tile([1, 128], BF)
            nc.gpsimd.memset(ones_t[:], 1.0)
            sixt_t = const_p.tile([128, 1], F32)
            nc.gpsimd.memset(sixt_t[:], 1.0 / H)

            # per-partition bias slices: [128, 8] with [p, fb] = b[fb*128 + p]
            bq_sb = const_p.tile([128, 8], F32)
            nc.sync.dma_start(bq_sb[:], bq_d[:].rearrange("(a p) -> p a", p=128))
            bk_sb = const_p.tile([128, 8], F32)
            nc.sync.dma_start(bk_sb[:], bk_d[:].rearrange("(a p) -> p a", p=128))
            bv_sb = const_p.tile([1, D], BF)
            nc.sync.dma_start(bv_sb[:], bv_d[:])

            # ---------------- Phase 1: LayerNorm (token-major) ----------------
            def layer_norm(src_dram, dst_dram, ntiles):
                for i in range(ntiles):
                    tx = in_p.tile([128, D], BF, tag="ln_in")
                    nc.sync.dma_start(tx[:], src_dram[i * 128:(i + 1) * 128, :])
                    st = stat_p.tile([128, 12], F32, tag="st12")
                    nc.vector.bn_stats(st[:, 0:6], tx[:, 0:512])
                    nc.vector.bn_stats(st[:, 6:12], tx[:, 512:1024])
                    mv = stat_p.tile([128, 2], F32, tag="mv")
                    nc.vector.bn_aggr(mv[:], st[:])
                    std = stat_p.tile([128, 1], F32, tag="std")
                    nc.scalar.activation(std[:], mv[:, 1:2], AF.Sqrt, bias=eps_t[:])
                    rinv = stat_p.tile([128, 1], F32, tag="rinv")
                    nc.vector.reciprocal(rinv[:], std[:])
                    th = hat_p.tile([128, D], BF, tag="hat")
                    nc.vector.tensor_scalar(
                        th[:], tx[:], mv[:, 0:1], rinv[:], ALU.subtract, ALU.mult
                    )
                    nc.sync.dma_start(dst_dram[i * 128:(i + 1) * 128, :], th[:])

            layer_norm(xt_d, that_dram, NW // 128)
            layer_norm(xa_d, ahat_dram, NV // 128)

            # ------------- Phase 2: reload transposed via DMA xbar -------------
            tT = []
            for db in range(8):
                t = tT_p.tile([128, NW], BF)
                nc.sync.dma_start(t[:], that_dram[:, db * 128:(db + 1) * 128],
                                  transpose=True)
                tT.append(t)
            aT = []
            for db in range(8):
                t = aT_p.tile([128, NV], BF)
                nc.sync.dma_start(t[:], ahat_dram[:, db * 128:(db + 1) * 128],
                                  transpose=True)
                aT.append(t)

            with tc.tile_pool(name="proj_ps", bufs=3, space="PSUM") as proj_ps:
                # ---------------- Phase 3a: q projection (qT[f, i]) ----------------
                qT = [None] * 8
                for fbg in range(2):
                    wts = []
                    for db in range(8):
                        w = w_p.tile([128, 512], BF, tag="w")
                        nc.sync.dma_start(
                            w[:], wq_d[db * 128:(db + 1) * 128,
                                       fbg * 512:(fbg + 1) * 512])
                        wts.append(w)
                    for f4 in range(4):
                        fb = fbg * 4 + f4
                        ps = proj_ps.tile([128, 512], F32)
                        for db in range(8):
                            nc.tensor.matmul(
                                ps[:], wts[db][:, f4 * 128:(f4 + 1) * 128],
                                tT[db][:], start=(db == 0), stop=(db == 7))
                        qt = qT_p.tile([128, NW], BF)
                        nc.vector.tensor_scalar_add(qt[:], ps[:], bq_sb[:, fb:fb + 1])
                        qT[fb] = qt

                # ---------------- Phase 3b: k projection (kT[f, j]) ----------------
                kT = [None] * 8
                for fbg in range(2):
                    wts = []
                    for db in range(8):
                        w = w_p.tile([128, 512], BF, tag="w")
                        nc.sync.dma_start(
                            w[:], wk_d[db * 128:(db + 1) * 128,
                                       fbg * 512:(fbg + 1) * 512])
                        wts.append(w)
                    for f4 in range(4):
                        fb = fbg * 4 + f4
                        kt = kT_p.tile([128, NV], BF)
                        for jh in range(2):
                            ps = proj_ps.tile([128, 512], F32)
                            for db in range(8):
                                nc.tensor.matmul(
                                    ps[:], wts[db][:, f4 * 128:(f4 + 1) * 128],
                                    aT[db][:, jh * 512:(jh + 1) * 512],
                                    start=(db == 0), stop=(db == 7))
                            nc.vector.tensor_scalar_add(
                                kt[:, jh * 512:(jh + 1) * 512], ps[:],
                                bk_sb[:, fb:fb + 1])
                        kT[fb] = kt

                # ------------- Phase 3c: v projection (token-major v[j, f]) -------------
                v = [v_p.tile([128, D], BF, tag="v", name=f"v{i}") for i in range(8)]
                for fh in range(2):
                    wts = []
                    for db in range(8):
                        w = w_p.tile([128, 512], BF, tag="w")
                        nc.sync.dma_start(
                            w[:], wv_d[db * 128:(db + 1) * 128,
                                       fh * 512:(fh + 1) * 512])
                        wts.append(w)
                    for jb in range(8):
                        ps = proj_ps.tile([128, 512], F32)
                        for db in range(8):
                            nc.tensor.matmul(
                                ps[:], aT[db][:, jb * 128:(jb + 1) * 128],
                                wts[db][:], start=(db == 0), stop=False)
                        nc.tensor.matmul(
                            ps[:], ones_t[:], bv_sb[0:1, fh * 512:(fh + 1) * 512],
                            start=False, stop=True)
                        nc.vector.tensor_copy(v[jb][:, fh * 512:(fh + 1) * 512], ps[:])

            # ---------------- Phase 4: attention ----------------
            with (
                tc.tile_pool(name="s_ps", bufs=4, space="PSUM") as s_ps,
                tc.tile_pool(name="o_ps", bufs=4, space="PSUM") as o_ps,
            ):
                outT = [outT_p.tile([128, NW], BF, tag="outT", name=f"outT{i}") for i in range(8)]
                lv = [[None] * 5 for _ in range(8)]
                alt = 0

                def mean_insert(jb, c):
                    nonlocal alt
                    k = 0
                    while lv[jb][k] is not None:
                        prev = lv[jb][k]
                        lv[jb][k] = None
                        nt = mean_p.tile([128, 512], BF, tag="mean")
                        eng = nc.vector if alt % 2 == 0 else nc.gpsimd
                        alt += 1
                        eng.tensor_add(nt[:], prev[:], c[:])
                        c = nt
                        k += 1
                    lv[jb][k] = c

                # head pairs (2*hp, 2*hp+1) share kT/qT tile hp; the two
                # heads occupy disjoint PE row groups (S) / col groups (out)
                # so adjacent matmuls overlap in the array.
                for hp in range(8):
                    fb = hp
                    pts0, pts1 = [], []
                    for jb in range(8):
                        sps0 = s_ps.tile([128, 512], F32, tag="sps")
                        nc.tensor.matmul(
                            sps0[:],
                            kT[fb][0:64, jb * 128:(jb + 1) * 128],
                            qT[fb][0:64, :], start=True, stop=True)
                        sps1 = s_ps.tile([128, 512], F32, tag="sps")
                        nc.tensor.matmul(
                            sps1[:],
                            kT[fb][64:128, jb * 128:(jb + 1) * 128],
                            qT[fb][64:128, :], start=True, stop=True)
                        pt0 = pt_p.tile([128, 512], BF, tag="pt")
                        nc.scalar.activation(pt0[:], sps0[:], AF.Sigmoid,
                                             bias=zero_t[:], scale=0.125)
                        pt1 = pt_p.tile([128, 512], BF, tag="pt")
                        nc.scalar.activation(pt1[:], sps1[:], AF.Sigmoid,
                                             bias=zero_t[:], scale=0.125)
                        pts0.append(pt0)
                        pts1.append(pt1)
                    ops0 = o_ps.tile([128, 512], F32, tag="ops")
                    ops1 = o_ps.tile([128, 512], F32, tag="ops")
                    h0, h1 = 2 * hp, 2 * hp + 1
                    for jb in range(8):
                        nc.tensor.matmul(
                            ops0[0:64, :], v[jb][:, h0 * 64:(h0 + 1) * 64],
                            pts0[jb][:], start=(jb == 0), stop=(jb == 7),
                            tile_position=(0, 0))
                        nc.tensor.matmul(
                            ops1[64:128, :], v[jb][:, h1 * 64:(h1 + 1) * 64],
                            pts1[jb][:], start=(jb == 0), stop=(jb == 7),
                            tile_position=(0, 64))
                    nc.scalar.activation(outT[fb][0:64, :], ops0[0:64, :], AF.Copy)
                    nc.scalar.activation(outT[fb][64:128, :], ops1[64:128, :], AF.Copy)
                    nc.sync.dma_start(
                        outT_dram[fb * 128:(fb + 1) * 128, :], outT[fb][:])
                    for jb in range(8):
                        mean_insert(jb, pts0[jb])
                        mean_insert(jb, pts1[jb])

                for jb in range(8):
                    amt = amT_p.tile([128, NW], BF)
                    nc.vector.tensor_scalar_mul(amt[:], lv[jb][4][:], sixt_t[:])
                    nc.sync.dma_start(amT_dram[jb * 128:(jb + 1) * 128, :], amt[:])

            # ------------- Phase 5: final transposes + cast stores -------------
            for ib in range(4):
                ot = fin_p.tile([128, D], BF, tag="fin")
                nc.sync.dma_start(ot[:], outT_dram[:, ib * 128:(ib + 1) * 128],
                                  transpose=True)
                nc.gpsimd.dma_start(out_d[ib * 128:(ib + 1) * 128, :], ot[:])
                at = fin_p.tile([128, NV], BF, tag="fin")
                nc.sync.dma_start(at[:], amT_dram[:, ib * 128:(ib + 1) * 128],
                                  transpose=True)
                nc.gpsimd.dma_start(am_d[ib * 128:(ib + 1) * 128, :], at[:])

    nc.compile()
    return nc


def _get_program():
    if "nc" not in _CACHE:
        _CACHE["nc"] = _build_program()
    return _CACHE["nc"]


def kernel(text, av_feat, tn_w, tn_b, an_w, an_b, Wq, bq, Wk, bk, Wv, bv,
           _trace=False):
    text = np.asarray(text, dtype=np.float32)
    av_feat = np.asarray(av_feat, dtype=np.float32)
    tn_w = np.asarray(tn_w, dtype=np.float32)
    tn_b = np.asarray(tn_b, dtype=np.float32)
    an_w = np.asarray(an_w, dtype=np.float32)
    an_b = np.asarray(an_b, dtype=np.float32)
    Wq = np.asarray(Wq, dtype=np.float32)
    bq = np.asarray(bq, dtype=np.float32)
    Wk = np.asarray(Wk, dtype=np.float32)
    bk = np.asarray(bk, dtype=np.float32)
    Wv = np.asarray(Wv, dtype=np.float32)
    bv = np.asarray(bv, dtype=np.float32)

    bs = text.shape[0]
    assert bs == NCORES and text.shape == (NCORES, NW, D)
    assert av_feat.shape == (NCORES, NV, D)

    # Fold LN affine into the projection weights (host-side, O(d^2)):
    #   q = ((x_hat*w + b) @ Wq.T + bq) = x_hat @ (Wq*w).T + (bq + Wq @ b)
    wqT = np.ascontiguousarray((Wq * tn_w[None, :]).T).astype(bf16)
    wkT = np.ascontiguousarray((Wk * an_w[None, :]).T).astype(bf16)
    wvT = np.ascontiguousarray((Wv * an_w[None, :]).T).astype(bf16)
    bq_eff = (bq + Wq @ tn_b).astype(np.float32)
    bk_eff = (bk + Wk @ an_b).astype(np.float32)
    bv_eff = (bv + Wv @ an_b).astype(bf16).reshape(1, D)

    nc = _get_program()

    in_maps = []
    for c in range(NCORES):
        in_maps.append({
            "xt": text[c].astype(bf16),
            "xa": av_feat[c].astype(bf16),
            "wqT": wqT, "wkT": wkT, "wvT": wvT,
            "bq": bq_eff, "bk": bk_eff, "bv": bv_eff,
        })

    res = run_bass_kernel_spmd(nc, in_maps, core_ids=list(range(NCORES)))
    out = np.stack([res.results[c]["out"] for c in range(NCORES)])
    am = np.stack([res.results[c]["am"] for c in range(NCORES)])
    return out, am


# revision 7
# speedup vs baseline: 1.0822x; 1.0822x over previous
"""Trainium2 Bass kernel for nn_CrossAttentionLayer (sigmoid cross-attention).

Sharding: pure data-parallel over the batch dim — core c computes batch c
(bs=8 across 8 NeuronCores, zero collectives).

Per-core device program (batch-local shapes: text (512,1024), av (1024,1024)):
  1. LayerNorm both streams in token-major layout (bn_stats/bn_aggr on DVE,
     rsqrt via ACT-Sqrt + DVE reciprocal, fused (x-mu)*rinv on DVE), bf16 out.
  2. Round-trip the normalized activations through DRAM and reload transposed
     via the DMA xbar (feature-major tiles for the matmul contraction dim).
  3. v projection first (token-major v, K=1 ones-matmul for its bias), then a
     fused loop over head pairs: q/k projection for the pair's feature block
     immediately followed by that pair's attention, so PE projection work
     overlaps ACT sigmoid work.
  4. Attention: S^T = kT_h^T @ qT_h (K=64, the two heads of a pair on disjoint
     PE row groups), kv-blocks paired into 2-bank PSUM tiles so each ACT
     sigmoid covers 1024 elements; out^T accumulated over kv with v stationary
     (pair on disjoint col groups via tile_position); attention-mean over
     heads via a pairwise bf16 add tree split across DVE and GpSimd.
  5. Outputs transposed back to token-major with PE transposes (spread through
     the loop for out, at the end for attn-mean) and SWDGE cast-stores.
"""
import numpy as np
import ml_dtypes

import concourse.bacc as bacc
import concourse.mybir as mybir
import concourse.tile as tile
from concourse.bass_utils import run_bass_kernel_spmd

bf16 = ml_dtypes.bfloat16
BF = mybir.dt.bfloat16
F32 = mybir.dt.float32
AF = mybir.ActivationFunctionType
ALU = mybir.AluOpType

NW = 512      # num_word (queries)
NV = 1024     # num_valid (keys/values)
D = 1024      # d_model
H = 16        # heads
DK = 64       # head dim
NCORES = 8

_CACHE: dict = {}


def _build_program():
    nc = bacc.Bacc("TRN2", target_bir_lowering=False, debug=False)

    xt_d = nc.declare_dram_parameter("xt", [NW, D], BF, isOutput=False)
    xa_d = nc.declare_dram_parameter("xa", [NV, D], BF, isOutput=False)
    wq_d = nc.declare_dram_parameter("wqT", [D, D], BF, isOutput=False)
    wk_d = nc.declare_dram_parameter("wkT", [D, D], BF, isOutput=False)
    wv_d = nc.declare_dram_parameter("wvT", [D, D], BF, isOutput=False)
    bq_d = nc.declare_dram_parameter("bq", [D], F32, isOutput=False)
    bk_d = nc.declare_dram_parameter("bk", [D], F32, isOutput=False)
    bv_d = nc.declare_dram_parameter("bv", [1, D], BF, isOutput=False)
    id_d = nc.declare_dram_parameter("ident", [128, 128], BF, isOutput=False)

    out_d = nc.declare_dram_parameter("out", [NW, D], F32, isOutput=True)
    am_d = nc.declare_dram_parameter("am", [NW, NV], F32, isOutput=True)

    that_dram = nc.dram_tensor("that_scratch", [NW, D], BF)
    ahat_dram = nc.dram_tensor("ahat_scratch", [NV, D], BF)

    with tile.TileContext(nc) as tc:
        import contextlib
        with contextlib.ExitStack() as ctx:
            const_p = ctx.enter_context(tc.tile_pool(name="const", bufs=1))
            in_p = ctx.enter_context(tc.tile_pool(name="in", bufs=3))
            stat_p = ctx.enter_context(tc.tile_pool(name="stat", bufs=24))
            hat_p = ctx.enter_context(tc.tile_pool(name="hat", bufs=3))
            tT_p = ctx.enter_context(tc.tile_pool(name="tT", bufs=8))
            aT_p = ctx.enter_context(tc.tile_pool(name="aT", bufs=8))
            w_p = ctx.enter_context(tc.tile_pool(name="w", bufs=22))
            wv_p = ctx.enter_context(tc.tile_pool(name="wv", bufs=10))
            qT_p = ctx.enter_context(tc.tile_pool(name="qT", bufs=3))
            kT_p = ctx.enter_context(tc.tile_pool(name="kT", bufs=3))
            v_p = ctx.enter_context(tc.tile_pool(name="v", bufs=8))
            pt_p = ctx.enter_context(tc.tile_pool(name="pt", bufs=10))
            mean_p = ctx.enter_context(tc.tile_pool(name="mean", bufs=22))
            otb_p = ctx.enter_context(tc.tile_pool(name="otb", bufs=3))
            row_p = ctx.enter_context(tc.tile_pool(name="row", bufs=8))
            amf_p = ctx.enter_context(tc.tile_pool(name="amf", bufs=4))

            eps_t = const_p.tile([128, 1], F32)
            nc.gpsimd.memset(eps_t[:], 1e-5)
            zero_t = const_p.tile([128, 1], F32)
            nc.gpsimd.memset(zero_t[:], 0.0)
            ones_t = const_p.tile([1, 128], BF)
            nc.gpsimd.memset(ones_t[:], 1.0)
            sixt_t = const_p.tile([128, 1], F32)
            nc.gpsimd.memset(sixt_t[:], 1.0 / H)
            ident = const_p.tile([128, 128], BF)
            nc.sync.dma_start(ident[:], id_d[:])

            # per-partition bias slices: [128, 8] with [p, fb] = b[fb*128 + p]
            bq_sb = const_p.tile([128, 8], F32)
            nc.sync.dma_start(bq_sb[:], bq_d[:].rearrange("(a p) -> p a", p=128))
            bk_sb = const_p.tile([128, 8], F32)
            nc.sync.dma_start(bk_sb[:], bk_d[:].rearrange("(a p) -> p a", p=128))
            bv_sb = const_p.tile([1, D], BF)
            nc.sync.dma_start(bv_sb[:], bv_d[:])

            # ---------------- Phase 1: LayerNorm (token-major) ----------------
            def layer_norm(src_dram, dst_dram, ntiles):
                for i in range(ntiles):
                    tx = in_p.tile([128, D], BF, tag="ln_in")
                    nc.sync.dma_start(tx[:], src_dram[i * 128:(i + 1) * 128, :])
                    st = stat_p.tile([128, 12], F32, tag="st12")
                    nc.vector.bn_stats(st[:, 0:6], tx[:, 0:512])
                    nc.vector.bn_stats(st[:, 6:12], tx[:, 512:1024])
                    mv = stat_p.tile([128, 2], F32, tag="mv")
                    nc.vector.bn_aggr(mv[:], st[:])
                    std = stat_p.tile([128, 1], F32, tag="std")
                    nc.scalar.activation(std[:], mv[:, 1:2], AF.Sqrt, bias=eps_t[:])
                    rinv = stat_p.tile([128, 1], F32, tag="rinv")
                    nc.vector.reciprocal(rinv[:], std[:])
                    th = hat_p.tile([128, D], BF, tag="hat")
                    nc.vector.tensor_scalar(
                        th[:], tx[:], mv[:, 0:1], rinv[:], ALU.subtract, ALU.mult
                    )
                    nc.sync.dma_start(dst_dram[i * 128:(i + 1) * 128, :], th[:])

            layer_norm(xa_d, ahat_dram, NV // 128)
            layer_norm(xt_d, that_dram, NW // 128)

            # ------------- Phase 2: reload transposed via DMA xbar -------------
            aT = []
            for db in range(8):
                t = aT_p.tile([128, NV], BF, tag="aT", name=f"aT{db}")
                nc.sync.dma_start(t[:], ahat_dram[:, db * 128:(db + 1) * 128],
                                  transpose=True)
                aT.append(t)
            tT = []
            for db in range(8):
                t = tT_p.tile([128, NW], BF, tag="tT", name=f"tT{db}")
                nc.sync.dma_start(t[:], that_dram[:, db * 128:(db + 1) * 128],
                                  transpose=True)
                tT.append(t)

            with (
                tc.tile_pool(name="work_ps", bufs=2, space="PSUM") as work_ps,
                tc.tile_pool(name="s_ps", bufs=2, space="PSUM") as s_ps,
                tc.tile_pool(name="o_ps", bufs=2, space="PSUM") as o_ps,
            ):
                # ------------- Phase 3: v projection (token-major v[j, f]) -------------
                v = [v_p.tile([128, D], BF, tag="v", name=f"v{i}") for i in range(8)]
                for fh in range(2):
                    wvs = []
                    for db in range(8):
                        w = wv_p.tile([128, 512], BF, tag="wv")
                        nc.sync.dma_start(
                            w[:], wv_d[db * 128:(db + 1) * 128,
                                       fh * 512:(fh + 1) * 512])
                        wvs.append(w)
                    for jb in range(8):
                        ps = work_ps.tile([128, 512], F32, tag="work")
                        for db in range(8):
                            nc.tensor.matmul(
                                ps[:], aT[db][:, jb * 128:(jb + 1) * 128],
                                wvs[db][:], start=(db == 0), stop=False)
                        nc.tensor.matmul(
                            ps[:], ones_t[:], bv_sb[0:1, fh * 512:(fh + 1) * 512],
                            start=False, stop=True)
                        nc.vector.tensor_copy(v[jb][:, fh * 512:(fh + 1) * 512], ps[:])

                # persistent token-major output accumulators
                out_row = [row_p.tile([128, D], BF, tag="row", name=f"orow{i}")
                           for i in range(4)]
                am_row = [row_p.tile([128, NV], BF, tag="row", name=f"arow{i}")
                          for i in range(4)]

                lv = [[None] * 5 for _ in range(4)]   # mean tree per kv pair
                alt = 0

                def mean_insert(jp, c):
                    nonlocal alt
                    k = 0
                    while lv[jp][k] is not None:
                        prev = lv[jp][k]
                        lv[jp][k] = None
                        nt = mean_p.tile([128, 1024], BF, tag="mean")
                        eng = nc.vector if alt % 2 == 0 else nc.gpsimd
                        alt += 1
                        eng.tensor_add(nt[:], prev[:], c[:])
                        c = nt
                        k += 1
                    lv[jp][k] = c

                # ---------- fused per-head-pair loop: projections + attention ----------
                for fb in range(8):
                    # q/k weight strips for this feature block: [d, fb*128 ±]
                    wqs, wks = [], []
                    for db in range(8):
                        wq = w_p.tile([128, 128], BF, tag="w")
                        nc.sync.dma_start(
                            wq[:], wq_d[db * 128:(db + 1) * 128,
                                        fb * 128:(fb + 1) * 128])
                        wqs.append(wq)
                        wk = w_p.tile([128, 128], BF, tag="w")
                        nc.sync.dma_start(
                            wk[:], wk_d[db * 128:(db + 1) * 128,
                                        fb * 128:(fb + 1) * 128])
                        wks.append(wk)

                    ps = work_ps.tile([128, 512], F32, tag="work")
                    for db in range(8):
                        nc.tensor.matmul(ps[:], wqs[db][:], tT[db][:],
                                         start=(db == 0), stop=(db == 7))
                    qt = qT_p.tile([128, NW], BF, tag="qt")
                    nc.vector.tensor_scalar_add(qt[:], ps[:], bq_sb[:, fb:fb + 1])

                    kt = kT_p.tile([128, NV], BF, tag="kt")
                    for jh in range(2):
                        ps = work_ps.tile([128, 512], F32, tag="work")
                        for db in range(8):
                            nc.tensor.matmul(
                                ps[:], wks[db][:],
                                aT[db][:, jh * 512:(jh + 1) * 512],
                                start=(db == 0), stop=(db == 7))
                        nc.vector.tensor_scalar_add(
                            kt[:, jh * 512:(jh + 1) * 512], ps[:],
                            bk_sb[:, fb:fb + 1])

                    # attention for heads (2*fb, 2*fb+1); kv blocks in pairs
                    h0, h1 = 2 * fb, 2 * fb + 1
                    pt0, pt1 = [], []
                    for jp in range(4):
                        je, jo = 2 * jp, 2 * jp + 1
                        for r0, plist in ((0, pt0), (64, pt1)):
                            sp = s_ps.tile([128, 1024], F32, tag="sp")
                            nc.tensor.matmul(
                                sp[:, 0:512],
                                kt[r0:r0 + 64, je * 128:(je + 1) * 128],
                                qt[r0:r0 + 64, :], start=True, stop=True)
                            nc.tensor.matmul(
                                sp[:, 512:1024],
                                kt[r0:r0 + 64, jo * 128:(jo + 1) * 128],
                                qt[r0:r0 + 64, :], start=True, stop=True)
                            pt = pt_p.tile([128, 1024], BF, tag="pt")
                            nc.scalar.activation(pt[:], sp[:], AF.Sigmoid,
                                                 bias=zero_t[:], scale=0.125)
                            plist.append(pt)

                    ops0 = o_ps.tile([128, 512], F32, tag="ops")
                    ops1 = o_ps.tile([128, 512], F32, tag="ops")
                    for jp in range(4):
                        for half in range(2):
                            jb = 2 * jp + half
                            nc.tensor.matmul(
                                ops0[0:64, :], v[jb][:, h0 * 64:(h0 + 1) * 64],
                                pt0[jp][:, half * 512:(half + 1) * 512],
                                start=(jb == 0), stop=(jb == 7),
                                tile_position=(0, 0))
                            nc.tensor.matmul(
                                ops1[64:128, :], v[jb][:, h1 * 64:(h1 + 1) * 64],
                                pt1[jp][:, half * 512:(half + 1) * 512],
                                start=(jb == 0), stop=(jb == 7),
                                tile_position=(0, 64))

                    otb = otb_p.tile([128, 512], BF, tag="otb")
                    nc.vector.tensor_copy(otb[0:64, :], ops0[0:64, :])
                    nc.vector.tensor_copy(otb[64:128, :], ops1[64:128, :])
                    for ib in range(4):
                        tp = work_ps.tile([128, 128], BF, tag="work")
                        nc.tensor.transpose(
                            tp[:], otb[:, ib * 128:(ib + 1) * 128], ident[:])
                        nc.scalar.activation(
                            out_row[ib][:, fb * 128:(fb + 1) * 128], tp[:],
                            AF.Copy)

                    for jp in range(4):
                        mean_insert(jp, pt0[jp])
                        mean_insert(jp, pt1[jp])

                # ---------------- attn-mean finalization ----------------
                for jp in range(4):
                    fin = amf_p.tile([128, 1024], BF, tag="amf")
                    nc.vector.tensor_scalar_mul(fin[:], lv[jp][4][:], sixt_t[:])
                    for half in range(2):
                        jb = 2 * jp + half
                        for ib in range(4):
                            tp = work_ps.tile([128, 128], BF, tag="work")
                            nc.tensor.transpose(
                                tp[:],
                                fin[:, half * 512 + ib * 128:
                                    half * 512 + (ib + 1) * 128],
                                ident[:])
                            nc.scalar.activation(
                                am_row[ib][:, jb * 128:(jb + 1) * 128], tp[:],
                                AF.Copy)

                for ib in range(4):
                    nc.gpsimd.dma_start(out_d[ib * 128:(ib + 1) * 128, :],
                                        out_row[ib][:])
                    nc.gpsimd.dma_start(am_d[ib * 128:(ib + 1) * 128, :],
                                        am_row[ib][:])

    nc.compile()
    return nc


def _get_program():
    if "nc" not in _CACHE:
        _CACHE["nc"] = _build_program()
    return _CACHE["nc"]


def kernel(text, av_feat, tn_w, tn_b, an_w, an_b, Wq, bq, Wk, bk, Wv, bv):
    text = np.asarray(text, dtype=np.float32)
    av_feat = np.asarray(av_feat, dtype=np.float32)
    tn_w = np.asarray(tn_w, dtype=np.float32)
    tn_b = np.asarray(tn_b, dtype=np.float32)
    an_w = np.asarray(an_w, dtype=np.float32)
    an_b = np.asarray(an_b, dtype=np.float32)
    Wq = np.asarray(Wq, dtype=np.float32)
    bq = np.asarray(bq, dtype=np.float32)
    Wk = np.asarray(Wk, dtype=np.float32)
    bk = np.asarray(bk, dtype=np.float32)
    Wv = np.asarray(Wv, dtype=np.float32)
    bv = np.asarray(bv, dtype=np.float32)

    bs = text.shape[0]
    assert bs == NCORES and text.shape == (NCORES, NW, D)
    assert av_feat.shape == (NCORES, NV, D)

    # Fold LN affine into the projection weights (host-side, O(d^2)):
    #   q = ((x_hat*w + b) @ Wq.T + bq) = x_hat @ (Wq*w).T + (bq + Wq @ b)
    wqT = np.ascontiguousarray((Wq * tn_w[None, :]).T).astype(bf16)
    wkT = np.ascontiguousarray((Wk * an_w[None, :]).T).astype(bf16)
    wvT = np.ascontiguousarray((Wv * an_w[None, :]).T).astype(bf16)
    bq_eff = (bq + Wq @ tn_b).astype(np.float32)
    bk_eff = (bk + Wk @ an_b).astype(np.float32)
    bv_eff = (bv + Wv @ an_b).astype(bf16).reshape(1, D)
    ident = np.eye(128).astype(bf16)

    nc = _get_program()

    in_maps = []
    for c in range(NCORES):
        in_maps.append({
            "xt": text[c].astype(bf16),
            "xa": av_feat[c].astype(bf16),
            "wqT": wqT, "wkT": wkT, "wvT": wvT,
            "bq": bq_eff, "bk": bk_eff, "bv": bv_eff,
            "ident": ident,
        })

    res = run_bass_kernel_spmd(nc, in_maps, core_ids=list(range(NCORES)))
    out = np.stack([res.results[c]["out"] for c in range(NCORES)])
    am = np.stack([res.results[c]["am"] for c in range(NCORES)])
    return out, am
